# revision 39
# baseline (speedup 1.0000x reference)
"""Trainium2 Bass kernel for nn_ExampleModel_9234179686517 (dense_mlp).

Model: bilinear grid-sample of a (4, 512, 512) featuremap at 4M points,
concat with xyz, then a 7->16->16->16->16->3 ReLU MLP.

Strategy (pure data parallel over 8 NeuronCores):
 - Host precomputes a bf16 "window table" qtab[y*64+xb] = the 16
   x-positions [8*xb, 8*xb+16) of border-clamped row pair (y, y+1), all
   4 channels, laid out (s, r, c) so that x-window selection is a
   contiguous slice: 16*2*4 bf16 = 256B per row, 32768 rows.
 - Each core gathers one 256B row per point with GPSIMD dma_gather,
   round-robined across all 4 SWDGE queues (4 SDMA engines drain in
   parallel; a single queue is 1-engine line-rate bound at ~30 GB/s).
 - The 3-bit sub-window x-position is resolved with conditional shifted
   copies (copy_predicated on int32 pairs) on VectorE, then x/y lerp.
 - TensorE transposes point-major -> feature-major and runs the MLP as
   block-diagonal (8 networks wide) bf16 matmuls with fp32 PSUM
   accumulation; ReLU+bias on ScalarE; results un-transposed on
   TensorE and DMAed back.
"""

import sys

for _p in ("/opt/trn_rl_repo", "/root/.axon_site/_ro/trn_rl_repo"):
    if _p not in sys.path:
        sys.path.insert(0, _p)

import numpy as np
import ml_dtypes

BF16 = ml_dtypes.bfloat16

N_TOTAL = 4_000_000
N_CORES = 8
C, H, W = 4, 512, 512
HID = 16

P = 128          # partitions
S = 512          # max slots per lane per coord tile
GS = 64          # slots per lane per MLP group (8192 points)
FPAD = 8         # padded feature count (3 xyz + 1 pad + 4 feat)
TCH = 64         # slots per gather chunk (8192 points)

N_CORE = N_TOTAL // N_CORES               # 500_000
M_SLOTS = 3968                            # slots per lane (mult of TCH, GS)
N_PAD = P * M_SLOTS                       # 507_904 padded points per core

NROWS = 512 * 64                          # window-table rows (= 32768)
NQ = 4                                    # SWDGE queues for gathers


def _build_host_constants(featuremap, Ws, bs):
    """Window table + block-diagonal bf16 weights."""
    fmT = np.ascontiguousarray(featuremap.transpose(1, 2, 0)).astype(np.float32)
    ys = np.arange(H)
    y2 = np.stack([ys, np.minimum(ys + 1, H - 1)], 1)            # [512, 2]
    xs = (np.arange(64)[:, None] * 8 + np.arange(16)[None, :])   # [64, 16]
    xs = np.minimum(xs, W - 1)
    # qtab[y, xb, s, r, c]: s-major so x-window selects are contiguous
    qtab = fmT[y2[:, None, None, :], xs[None, :, :, None], :]    # [512,64,16,2,4]
    qtab = qtab.reshape(NROWS, 128).astype(BF16)

    W1, W2, W3, W4, W5 = Ws
    b1, b2, b3, b4, b5 = bs

    # stg feature order: (x, y, z, pad, f0..f3)
    W1a = np.zeros((FPAD, HID), np.float32)
    W1a[0:3] = W1[0:3]
    W1a[4:8] = W1[3:7]

    def blockdiag(Wm, nb):
        fi, fo = Wm.shape
        out = np.zeros((fi * nb, fo * nb), np.float32)
        for b in range(nb):
            out[b * fi:(b + 1) * fi, b * fo:(b + 1) * fo] = Wm
        return out

    w1blk = blockdiag(W1a, 8)                      # [64, 128]
    w1stack = np.concatenate([w1blk, w1blk], 0)    # [128, 128]

    return {
        "qtab": qtab,
        "w1stack": w1stack.astype(BF16),
        "w2blk": blockdiag(W2, 8).astype(BF16),
        "w3blk": blockdiag(W3, 8).astype(BF16),
        "w4blk": blockdiag(W4, 8).astype(BF16),
        "w5blk": blockdiag(W5, 8).astype(BF16),
        "b1blk": np.tile(b1, 8).reshape(P, 1).astype(np.float32),
        "b2blk": np.tile(b2, 8).reshape(P, 1).astype(np.float32),
        "b3blk": np.tile(b3, 8).reshape(P, 1).astype(np.float32),
        "b4blk": np.tile(b4, 8).reshape(P, 1).astype(np.float32),
        "b5blk": np.tile(b5, 8).reshape(24, 1).astype(np.float32),
        "id128": np.eye(P, dtype=np.float32).astype(BF16),
        "id24": np.eye(24, dtype=np.float32),
    }


def build_program(n_slots=M_SLOTS, s_tile=S, mlp=True, gather=True):
    """Build the per-core Bass program (same program for all 8 cores)."""
    import concourse.bass as bass
    import concourse.tile as tile
    from concourse import bacc, mybir

    f32 = mybir.dt.float32
    bf16 = mybir.dt.bfloat16
    i16 = mybir.dt.int16
    i32 = mybir.dt.int32
    u8d = mybir.dt.uint8
    AF = mybir.ActivationFunctionType
    OP = mybir.AluOpType

    assert n_slots % TCH == 0 and s_tile % GS == 0 and s_tile % TCH == 0
    n_pad = P * n_slots
    GNI = 1024                       # idxs per dma_gather call (64 descs per
                                     # engine-lane = the single-packet limit)
    gcalls = P * TCH // GNI          # gather calls per select chunk (8)
    TW = P * TCH // 16               # wrap idx columns per chunk (512)

    # iteration schedule: small head tiles (short ramp to first gather),
    # full s_tile iters, then a progressively-smaller tail (short drain
    # after the final gather)
    tiles = []
    off = 0
    if n_slots > 3 * s_tile and s_tile >= 4 * TCH:
        for st in (TCH, TCH, s_tile - 2 * TCH):
            tiles.append((off, st))
            off += st
    while off < n_slots:
        st = min(s_tile, n_slots - off)
        if n_slots - off == st and st < s_tile and st > 2 * GS:
            # tail: progressively smaller iterations so the pipeline
            # drain after the final gather is minimal
            while st > 2 * GS:
                big = st - 2 * GS
                big -= big % GS
                if big < GS:
                    break
                tiles.append((off, big))
                off += big
                st -= big
            tiles.append((off, GS))
            tiles.append((off + GS, GS))
            off += 2 * GS
            continue
        assert st % TCH == 0 and st % GS == 0
        tiles.append((off, st))
        off += st
    assert off == n_slots, (off, n_slots)

    nc = bacc.Bacc("TRN2", target_bir_lowering=False, debug=False,
                   enable_asserts=False, num_devices=N_CORES,
                   num_swdge_queues=NQ)

    xin = nc.dram_tensor("x", [n_pad, 3], f32, kind="ExternalInput").ap()
    qtab = nc.dram_tensor("qtab", [NROWS, 128], bf16, kind="ExternalInput").ap()
    w1stack = nc.dram_tensor("w1stack", [P, P], bf16, kind="ExternalInput").ap()
    w2 = nc.dram_tensor("w2blk", [P, P], bf16, kind="ExternalInput").ap()
    w3 = nc.dram_tensor("w3blk", [P, P], bf16, kind="ExternalInput").ap()
    w4 = nc.dram_tensor("w4blk", [P, P], bf16, kind="ExternalInput").ap()
    w5 = nc.dram_tensor("w5blk", [P, 24], bf16, kind="ExternalInput").ap()
    b1 = nc.dram_tensor("b1blk", [P, 1], f32, kind="ExternalInput").ap()
    b2i = nc.dram_tensor("b2blk", [P, 1], f32, kind="ExternalInput").ap()
    b3i = nc.dram_tensor("b3blk", [P, 1], f32, kind="ExternalInput").ap()
    b4i = nc.dram_tensor("b4blk", [P, 1], f32, kind="ExternalInput").ap()
    b5i = nc.dram_tensor("b5blk", [24, 1], f32, kind="ExternalInput").ap()
    id128 = nc.dram_tensor("id128", [P, P], bf16, kind="ExternalInput").ap()
    id24 = nc.dram_tensor("id24", [24, 24], f32, kind="ExternalInput").ap()
    yout = nc.dram_tensor("y", [n_pad, 3], f32, kind="ExternalOutput").ap()

    # lane p owns rows [p*n_slots, (p+1)*n_slots)  (contiguous HBM runs)
    xv = xin.rearrange("(p s) c -> p s c", p=P)
    yv = yout.rearrange("(p s) c -> p s c", p=P)

    BIGF = float(2 ** 23)

    from contextlib import ExitStack

    with tile.TileContext(nc) as tc, ExitStack() as ctx:
            ep = ctx.enter_context
            consts = ep(tc.tile_pool(name="consts", bufs=1))
            xio = ep(tc.tile_pool(name="xio", bufs=2))
            xwp = ep(tc.tile_pool(name="xw", bufs=1))
            coord2 = ep(tc.tile_pool(name="coord2", bufs=2))
            coord1 = ep(tc.tile_pool(name="coord1", bufs=1))
            wcoord = ep(tc.tile_pool(name="wcoord", bufs=1))
            jidxp = ep(tc.tile_pool(name="jidx", bufs=8))
            gatp = ep(tc.tile_pool(name="gat", bufs=4))
            shiftp = ep(tc.tile_pool(name="shift", bufs=2))
            stagep = ep(tc.tile_pool(name="stage", bufs=2))
            tsbp = ep(tc.tile_pool(name="tsb", bufs=2))
            actsp = ep(tc.tile_pool(name="acts", bufs=3))
            s5p = ep(tc.tile_pool(name="s5", bufs=2))
            ostagep = ep(tc.tile_pool(name="ostage", bufs=4))
            ptr = ep(tc.tile_pool(name="ptr", bufs=2, space="PSUM"))
            pmm = ep(tc.tile_pool(name="pmm", bufs=2, space="PSUM"))
            p5 = ep(tc.tile_pool(name="p5", bufs=1, space="PSUM"))
            dramp = ep(tc.tile_pool(name="dram", bufs=2, space="DRAM"))

            # ---- constants into SBUF
            w1_sb = consts.tile([P, P], bf16, tag="w1")
            w2_sb = consts.tile([P, P], bf16, tag="w2")
            w3_sb = consts.tile([P, P], bf16, tag="w3")
            w4_sb = consts.tile([P, P], bf16, tag="w4")
            w5_sb = consts.tile([P, 24], bf16, tag="w5")
            b1_sb = consts.tile([P, 1], f32, tag="b1")
            b2_sb = consts.tile([P, 1], f32, tag="b2")
            b3_sb = consts.tile([P, 1], f32, tag="b3")
            b4_sb = consts.tile([P, 1], f32, tag="b4")
            b5_sb = consts.tile([24, 1], f32, tag="b5")
            id128_sb = consts.tile([P, P], bf16, tag="id128")
            id24_sb = consts.tile([24, 24], f32, tag="id24")
            cm05 = consts.tile([P, 1], f32, tag="cm05")
            nc.vector.memset(cm05[:], -0.5)
            ones_sb = consts.tile([P, s_tile], u8d, tag="ones")
            nc.vector.memset(ones_sb[:], 1)
            _const_dmas = (
                (w1_sb, w1stack), (w2_sb, w2), (w3_sb, w3), (w4_sb, w4),
                (w5_sb, w5), (b1_sb, b1), (b2_sb, b2i), (b3_sb, b3i),
                (b4_sb, b4i), (b5_sb, b5i), (id128_sb, id128), (id24_sb, id24),
            )

            def floor_exact(pool, fsrc, tagp, full_shape=None):
                """returns AP with floor(fsrc); exact for f in [0, 2^22)."""
                pk, fw = fsrc.shape[0], fsrc.shape[1]
                fs = full_shape or [P, s_tile]
                b_ = pool.tile(fs, f32, tag=f"fb{tagp}", name=f"fb{tagp}")[:pk, :fw]
                nc.vector.tensor_scalar(out=b_, in0=fsrc, scalar1=BIGF,
                                        scalar2=BIGF, op0=OP.add, op1=OP.subtract)
                # comparison temp shared across call sites (uses are disjoint)
                cgt = pool.tile(fs, f32, tag="fcS", name="fcS")[:pk, :fw]
                nc.vector.tensor_tensor(out=cgt, in0=b_, in1=fsrc, op=OP.is_gt)
                nc.vector.tensor_tensor(out=b_, in0=b_, in1=cgt, op=OP.subtract)
                return b_

            qctr = 0  # global gather queue round-robin
            gni_reg = nc.gpsimd.snap(GNI)

            # ======== PROLOGUE: all wrap-layout gather idxs -> DRAM =====
            # partition (ch, q) of the stacked tile holds chunk ch's
            # 16-partition wrap rows; one 128-wide compute pass per s_tile.
            # Precomputing all iterations up front keeps the Pool engine's
            # gather stream free of mid-loop jidx stalls.
            jd_list = [dramp.tile([P, TW], i16, tag=f"jd{i}",
                                       name=f"jd{i}", bufs=1)
                       for i in range(len(tiles))]
            for it, (sl0, st) in enumerate(tiles):
                chunks = st // TCH
                pk = 16 * chunks
                xws = xwp.tile([P, TW, 3], f32, tag="xws", name="xws")
                for ch in range(chunks):
                    xw_src = bass.AP(
                        tensor=xin.tensor,
                        offset=(sl0 + ch * TCH) * 3,
                        ap=[[n_slots * 3, 16], [16 * n_slots * 3, 8],
                            [3, TCH], [1, 3]],
                    )
                    nc.sync.dma_start(
                        out=xws[16 * ch:16 * (ch + 1)].rearrange(
                            "p (tl s) c -> p tl s c", tl=8),
                        in_=xw_src)
                fxw = wcoord.tile([P, TW], f32, tag="fxw", name="fxw")[:pk]
                nc.scalar.activation(out=fxw, in_=xws[:pk, :, 0], func=AF.Relu,
                                     bias=cm05[:pk], scale=float(W))
                fyw = wcoord.tile([P, TW], f32, tag="fyw", name="fyw")[:pk]
                nc.scalar.activation(out=fyw, in_=xws[:pk, :, 1], func=AF.Relu,
                                     bias=cm05[:pk], scale=float(H))
                u8w = wcoord.tile([P, TW], f32, tag="u8w", name="u8w")[:pk]
                nc.vector.tensor_scalar(out=u8w, in0=fxw, scalar1=0.125,
                                        scalar2=None, op0=OP.mult)
                xbw = floor_exact(wcoord, u8w, "xw", full_shape=[P, TW])
                iyw = floor_exact(wcoord, fyw, "yw", full_shape=[P, TW])
                idxf = wcoord.tile([P, TW], f32, tag="idxf", name="idxf")[:pk]
                nc.vector.scalar_tensor_tensor(out=idxf, in0=iyw, scalar=64.0,
                                               in1=xbw, op0=OP.mult, op1=OP.add)
                # cast to int16, permuting t'' = (tl, s) -> t = 8*s + tl
                jidx16 = wcoord.tile([P, TW], i16, tag="jidx16",
                                     name="jidx16")[:pk]
                nc.vector.tensor_copy(
                    out=jidx16.rearrange("p (s tl) -> p tl s", tl=8),
                    in_=idxf.rearrange("p (tl s) -> p tl s", s=TCH))
                nc.sync.dma_start(out=jd_list[it][:pk], in_=jidx16)

            for sb, csrc in _const_dmas:
                nc.sync.dma_start(out=sb[:], in_=csrc)

            for it, (sl0, st) in enumerate(tiles):
                chunks = st // TCH
                groups = st // GS

                # ======== main (point-layout) coordinate pipeline ========
                xt = xio.tile([P, s_tile, 3], f32, tag="xt", name="xt")[:, :st]
                nc.sync.dma_start(out=xt, in_=xv[:, sl0:sl0 + st, :])

                fx = coord1.tile([P, s_tile], f32, tag="fx", name="fx")[:, :st]
                nc.scalar.activation(out=fx, in_=xt[:, :, 0], func=AF.Relu,
                                     bias=cm05[:], scale=float(W))
                fy = coord1.tile([P, s_tile], f32, tag="fy", name="fy")[:, :st]
                nc.scalar.activation(out=fy, in_=xt[:, :, 1], func=AF.Relu,
                                     bias=cm05[:], scale=float(H))

                u8 = coord1.tile([P, s_tile], f32, tag="u8", name="u8")[:, :st]
                nc.vector.tensor_scalar(out=u8, in0=fx, scalar1=0.125,
                                        scalar2=None, op0=OP.mult)
                xbf = floor_exact(coord1, u8, "x")
                u = coord1.tile([P, s_tile], f32, tag="u", name="u")[:, :st]
                nc.vector.scalar_tensor_tensor(out=u, in0=xbf, scalar=-8.0,
                                               in1=fx, op0=OP.mult, op1=OP.add)
                # q bits (f32 masks for predicated selects) + wx
                b2f = coord2.tile([P, s_tile], f32, tag="b2f", name="b2f")[:, :st]
                nc.vector.tensor_scalar(out=b2f, in0=u, scalar1=4.0,
                                        scalar2=None, op0=OP.is_ge)
                u2 = coord1.tile([P, s_tile], f32, tag="u2", name="u2")[:, :st]
                nc.vector.scalar_tensor_tensor(out=u2, in0=b2f, scalar=-4.0,
                                               in1=u, op0=OP.mult, op1=OP.add)
                b1f = coord1.tile([P, s_tile], f32, tag="b1f", name="b1f")[:, :st]
                nc.vector.tensor_scalar(out=b1f, in0=u2, scalar1=2.0,
                                        scalar2=None, op0=OP.is_ge)
                u3 = coord1.tile([P, s_tile], f32, tag="u3", name="u3")[:, :st]
                nc.vector.scalar_tensor_tensor(out=u3, in0=b1f, scalar=-2.0,
                                               in1=u2, op0=OP.mult, op1=OP.add)
                b0f = coord2.tile([P, s_tile], f32, tag="b0f", name="b0f")[:, :st]
                nc.vector.tensor_scalar(out=b0f, in0=u3, scalar1=1.0,
                                        scalar2=None, op0=OP.is_ge)
                wx = coord2.tile([P, s_tile], bf16, tag="wx", name="wx")[:, :st]
                nc.vector.tensor_tensor(out=wx, in0=u3, in1=b0f, op=OP.subtract)
                # monotone ge-masks for the 4-way first select stage (f32
                # 0.0/1.0, used bitcast-as-i32: 1.0f is a nonzero pattern)
                ge2 = coord2.tile([P, s_tile], f32, tag="ge2", name="ge2")[:, :st]
                nc.vector.tensor_scalar(out=ge2, in0=u, scalar1=2.0,
                                        scalar2=None, op0=OP.is_ge)
                ge6 = coord2.tile([P, s_tile], f32, tag="ge6", name="ge6")[:, :st]
                nc.vector.tensor_scalar(out=ge6, in0=u, scalar1=6.0,
                                        scalar2=None, op0=OP.is_ge)

                iyf = floor_exact(coord1, fy, "y")
                wy = coord2.tile([P, s_tile], bf16, tag="wy", name="wy")[:, :st]
                nc.vector.tensor_tensor(out=wy, in0=fy, in1=iyf, op=OP.subtract)

                # jidx: read back replicated to all 8 Q7 core groups (DRAM
                # source APs may have 0-step partition dims); one tile per
                # chunk so each gather only waits on its own slice
                jidx_ch = []
                for ch in range(chunks):
                    jslc = jd_list[it][16 * ch:16 * (ch + 1), :]
                    rep_src = bass.AP(tensor=jslc.tensor, offset=jslc.offset,
                                      ap=[[0, 8]] + list(jslc.ap))
                    jt = jidxp.tile([P, TW], i16, tag="jidx")
                    nc.sync.dma_start(out=jt[:], in_=rep_src)
                    jidx_ch.append(jt)

                # ======== gather + select + lerp per chunk ========
                stg = stagep.tile([P, s_tile, FPAD], bf16, tag="stg", name="stg")[:, :st]
                nc.vector.memset(stg[:, :, 3], 0.0)
                nc.scalar.activation(out=stg[:, :, 0:3], in_=xt,
                                     func=AF.Copy, bias=0.0, scale=1.0)

                # chunks are processed in pairs: both chunks' gathers are
                # issued back-to-back, then both select ladders run -- this
                # halves the Pool<->Vector handshake frequency.
                chpairs = [[c for c in (cb, cb + 1) if c < chunks]
                           for cb in range(0, chunks, 2)]
                for chp in chpairs:
                  G_of = {}
                  for ch in chp:
                    G_of[ch] = gatp.tile([P, TCH, 128], bf16, tag="G", name="G")
                    if gather:
                        gsl = GNI // P       # slots per gather call (8)
                        gw = GNI // 16       # idx cols per gather call (64)
                        for k in range(gcalls):
                            nc.gpsimd.dma_gather(
                                out_ap=G_of[ch][:, k * gsl:(k + 1) * gsl, :],
                                in_ap=qtab,
                                idxs_ap=jidx_ch[ch][:, k * gw:(k + 1) * gw],
                                num_idxs=GNI, num_idxs_reg=gni_reg, elem_size=128,
                                single_packet=(GNI <= 1024),
                                queue_num=qctr % NQ)
                            qctr += 1
                    else:
                        nc.vector.memset(G_of[ch][:], 0.25)

                  for ch in chp:
                    cs = ch * TCH
                    G = G_of[ch]
                    # payload layout (s, r, c): x-window selects are
                    # contiguous slices. 4-way first stage via a monotone
                    # is_ge mask cascade (last true predicate wins), then a
                    # 2-way second stage; all copies on int32 pairs.
                    onesv = ones_sb[:, 0:TCH, None]
                    g2v = ge2.bitcast(i32)[:, cs:cs + TCH, None]
                    g4v = b2f.bitcast(i32)[:, cs:cs + TCH, None]
                    g6v = ge6.bitcast(i32)[:, cs:cs + TCH, None]
                    m0v = b0f.bitcast(i32)[:, cs:cs + TCH, None]

                    W2t = shiftp.tile([P, TCH, 24], bf16, tag="W2")
                    nc.vector.copy_predicated(
                        out=W2t[:].bitcast(i32),
                        mask=onesv.to_broadcast([P, TCH, 12]),
                        data=G[:].bitcast(i32)[:, :, 0:12])
                    nc.vector.copy_predicated(
                        out=W2t[:].bitcast(i32),
                        mask=g2v.to_broadcast([P, TCH, 12]),
                        data=G[:].bitcast(i32)[:, :, 8:20])
                    nc.vector.copy_predicated(
                        out=W2t[:].bitcast(i32),
                        mask=g4v.to_broadcast([P, TCH, 12]),
                        data=G[:].bitcast(i32)[:, :, 16:28])
                    nc.vector.copy_predicated(
                        out=W2t[:].bitcast(i32),
                        mask=g6v.to_broadcast([P, TCH, 12]),
                        data=G[:].bitcast(i32)[:, :, 24:36])
                    W3t = shiftp.tile([P, TCH, 16], bf16, tag="W3")
                    nc.vector.copy_predicated(
                        out=W3t[:].bitcast(i32),
                        mask=onesv.to_broadcast([P, TCH, 8]),
                        data=W2t[:].bitcast(i32)[:, :, 0:8])
                    nc.vector.copy_predicated(
                        out=W3t[:].bitcast(i32),
                        mask=m0v.to_broadcast([P, TCH, 8]),
                        data=W2t[:].bitcast(i32)[:, :, 4:12])

                    # lerp x then y -> staging features (all contiguous)
                    wxv = wx[:, cs:cs + TCH, None].to_broadcast([P, TCH, 8])
                    wyv = wy[:, cs:cs + TCH, None].to_broadcast([P, TCH, 4])
                    d = shiftp.tile([P, TCH, 8], bf16, tag="d")
                    nc.vector.tensor_tensor(out=d[:], in0=W3t[:, :, 8:16],
                                            in1=W3t[:, :, 0:8], op=OP.subtract)
                    nc.vector.tensor_tensor(out=d[:], in0=d[:], in1=wxv, op=OP.mult)
                    nc.vector.tensor_tensor(out=d[:], in0=W3t[:, :, 0:8],
                                            in1=d[:], op=OP.add)
                    e = shiftp.tile([P, TCH, 4], bf16, tag="e")
                    nc.vector.tensor_tensor(out=e[:], in0=d[:, :, 4:8],
                                            in1=d[:, :, 0:4], op=OP.subtract)
                    nc.vector.tensor_tensor(out=e[:], in0=e[:], in1=wyv, op=OP.mult)
                    nc.vector.tensor_tensor(out=stg[:, cs:cs + TCH, 4:8],
                                            in0=d[:, :, 0:4], in1=e[:], op=OP.add)

                stg_flat = stg.rearrange("p s f -> p (s f)")

                if not mlp:
                    ost = ostagep.tile([P, s_tile, 3], f32, tag="ostd",
                                       name="ostd")[:, :st]
                    nc.scalar.activation(out=ost, in_=stg[:, :, 4:7],
                                         func=AF.Copy, bias=0.0, scale=1.0)
                    nc.sync.dma_start(out=yv[:, sl0:sl0 + st, :], in_=ost)
                    continue

                # ======== MLP groups (GS slots = 8192 points each) ========
                # Groups are processed in pairs with layers interleaved so
                # each activation issues right after the OTHER group's
                # matmul -- the ~2us PE->ACT semaphore latency hides behind
                # the sibling's work instead of stalling the Scalar stream.
                for gp in range(0, groups, 2):
                    gpair = [g for g in (gp, gp + 1) if g < groups]
                    tsb_of = {}
                    for g in gpair:
                        t_ps = ptr.tile([P, 4, P], bf16, tag="tp")
                        for c4 in range(4):
                            base = (g * GS + c4 * 16) * FPAD
                            nc.tensor.transpose(out=t_ps[:, c4, :],
                                                in_=stg_flat[:, base:base + P],
                                                identity=id128_sb[:])
                        tsb_of[g] = tsbp.tile([P, 4, P], bf16, tag="tsb", name="tsb")
                        nc.scalar.activation(out=tsb_of[g][:], in_=t_ps[:],
                                             func=AF.Copy, bias=0.0, scale=1.0)

                    # L1: the 4 c4-blocks are contiguous in both rhs and
                    # psum -> one wide matmul per 64-row half.
                    ps_of = {}
                    for g in gpair:
                        ps = pmm.tile([P, 1024], f32, tag="ps", name="ps")
                        for half in range(2):
                            nc.tensor.matmul(
                                out=ps[:, half * 512:(half + 1) * 512],
                                lhsT=w1_sb[half * 64:(half + 1) * 64, :],
                                rhs=tsb_of[g][half * 64:(half + 1) * 64].rearrange(
                                    "p c4 l -> p (c4 l)"),
                                start=True, stop=True)
                        ps_of[g] = ps
                    h_of = {}
                    for g in gpair:
                        h_of[g] = actsp.tile([P, 1024], bf16, tag="h", name="h")
                        nc.scalar.activation(out=h_of[g][:], in_=ps_of[g][:],
                                             func=AF.Relu, bias=b1_sb[:], scale=1.0)

                    for w_sb, bias_sb in ((w2_sb, b2_sb), (w3_sb, b3_sb), (w4_sb, b4_sb)):
                        for g in gpair:
                            ps = pmm.tile([P, 1024], f32, tag="ps", name="ps")
                            nc.tensor.matmul(out=ps[:, 0:512], lhsT=w_sb[:],
                                             rhs=h_of[g][:, 0:512],
                                             start=True, stop=True)
                            nc.tensor.matmul(out=ps[:, 512:1024], lhsT=w_sb[:],
                                             rhs=h_of[g][:, 512:1024],
                                             start=True, stop=True)
                            ps_of[g] = ps
                        for g in gpair:
                            h_of[g] = actsp.tile([P, 1024], bf16, tag="h", name="h")
                            nc.scalar.activation(out=h_of[g][:], in_=ps_of[g][:],
                                                 func=AF.Relu, bias=bias_sb[:],
                                                 scale=1.0)

                    s5_of = {}
                    for g in gpair:
                        ps5 = p5.tile([24, 1024], f32, tag="ps5", name="ps5")
                        nc.tensor.matmul(out=ps5[:, 0:512], lhsT=w5_sb[:],
                                         rhs=h_of[g][:, 0:512],
                                         start=True, stop=True)
                        nc.tensor.matmul(out=ps5[:, 512:1024], lhsT=w5_sb[:],
                                         rhs=h_of[g][:, 512:1024],
                                         start=True, stop=True)
                        s5_of[g] = s5p.tile([24, 1024], f32, tag="s5", name="s5")
                        nc.scalar.activation(out=s5_of[g][:], in_=ps5[:],
                                             func=AF.Identity, bias=b5_sb[:],
                                             scale=1.0)

                    for g in gpair:
                        s5 = s5_of[g]
                        u_ps = ptr.tile([P, 8, 24], f32, tag="tp")
                        for ui in range(2):
                            for c4 in range(4):
                                nc.tensor.transpose(
                                    out=u_ps[:, c4 * 2 + ui, :],
                                    in_=s5[:, ui * 512 + c4 * P: ui * 512 + (c4 + 1) * P],
                                    identity=id24_sb[:])
                        uv = u_ps.rearrange("p k (b c) -> p k b c", c=3)
                        ost = ostagep.tile([P, GS, 3], f32, tag="ost", name="ost")
                        ostg = ost.rearrange("p (k b) d -> p k b d", k=8)
                        nc.scalar.activation(out=ostg, in_=uv,
                                             func=AF.Copy, bias=0.0, scale=1.0)
                        # issue the store from the Scalar sequencer (HWDGE on
                        # TRN2) so it never head-of-line blocks the Sync
                        # queue's jidx prefetches behind the MLP
                        nc.scalar.dma_start(
                            out=yv[:, sl0 + g * GS:sl0 + (g + 1) * GS, :], in_=ost)

    nc.compile()
    return nc


_PROGRAM_CACHE = {}


def _get_program(n_slots, s_tile):
    key = (n_slots, s_tile)
    if key not in _PROGRAM_CACHE:
        _PROGRAM_CACHE[key] = build_program(n_slots, s_tile)
    return _PROGRAM_CACHE[key]


def make_in_maps(x_full, consts, n_slots=M_SLOTS, n_cores=N_CORES):
    n_pad = P * n_slots
    per = x_full.shape[0] // n_cores
    in_maps = []
    for c in range(n_cores):
        xpad = np.zeros((n_pad, 3), np.float32)
        xpad[:per] = x_full[c * per:(c + 1) * per]
        in_maps.append({"x": xpad, **{k: np.ascontiguousarray(v)
                                      for k, v in consts.items()}})
    return in_maps


def kernel(**inputs):
    from concourse import bass_utils
    from concourse.bass_interp import get_hw_module

    x = np.asarray(inputs["x"], dtype=np.float32)
    fm = np.asarray(inputs["featuremap"], dtype=np.float32)
    Ws = [np.asarray(inputs[f"W{i}"], dtype=np.float32) for i in range(1, 6)]
    bs = [np.asarray(inputs[f"b{i}"], dtype=np.float32) for i in range(1, 6)]

    consts = _build_host_constants(fm, Ws, bs)
    n = x.shape[0]
    assert n == N_TOTAL, n
    per = n // N_CORES

    nc = _get_program(M_SLOTS, S)
    old_m = nc.m
    nc.m = get_hw_module(nc.m)
    try:
        in_maps = make_in_maps(x, consts)
        res = bass_utils.run_bass_kernel_spmd(nc, in_maps,
                                              core_ids=list(range(N_CORES)))
    finally:
        nc.m = old_m
    outs = [r["y"][:per] for r in res.results]
    return np.concatenate(outs, axis=0).astype(np.float32)


if __name__ == "__main__":
    build_program(256, 128)
    print("small program built OK")



# revision 40
# speedup vs baseline: 1.0005x; 1.0005x over previous
"""Trainium2 Bass kernel for nn_ExampleModel_9234179686517 (dense_mlp).

Model: bilinear grid-sample of a (4, 512, 512) featuremap at 4M points,
concat with xyz, then a 7->16->16->16->16->3 ReLU MLP.

Strategy (pure data parallel over 8 NeuronCores):
 - Host precomputes a bf16 "window table" qtab[y*64+xb] = the 16
   x-positions [8*xb, 8*xb+16) of border-clamped row pair (y, y+1), all
   4 channels, laid out (s, r, c) so that x-window selection is a
   contiguous slice: 16*2*4 bf16 = 256B per row, 32768 rows.
 - Each core gathers one 256B row per point with GPSIMD dma_gather,
   round-robined across all 4 SWDGE queues (4 SDMA engines drain in
   parallel; a single queue is 1-engine line-rate bound at ~30 GB/s).
 - The 3-bit sub-window x-position is resolved with conditional shifted
   copies (copy_predicated on int32 pairs) on VectorE, then x/y lerp.
 - TensorE transposes point-major -> feature-major and runs the MLP as
   block-diagonal (8 networks wide) bf16 matmuls with fp32 PSUM
   accumulation; ReLU+bias on ScalarE; results un-transposed on
   TensorE and DMAed back.
"""

import sys

for _p in ("/opt/trn_rl_repo", "/root/.axon_site/_ro/trn_rl_repo"):
    if _p not in sys.path:
        sys.path.insert(0, _p)

import numpy as np
import ml_dtypes

BF16 = ml_dtypes.bfloat16

N_TOTAL = 4_000_000
N_CORES = 8
C, H, W = 4, 512, 512
HID = 16

P = 128          # partitions
S = 512          # max slots per lane per coord tile
GS = 64          # slots per lane per MLP group (8192 points)
FPAD = 8         # padded feature count (3 xyz + 1 pad + 4 feat)
TCH = 64         # slots per gather chunk (8192 points)

N_CORE = N_TOTAL // N_CORES               # 500_000
M_SLOTS = 3968                            # slots per lane (mult of TCH, GS)
N_PAD = P * M_SLOTS                       # 507_904 padded points per core

NROWS = 512 * 64                          # window-table rows (= 32768)
NQ = 4                                    # SWDGE queues for gathers


def _build_host_constants(featuremap, Ws, bs):
    """Window table + block-diagonal bf16 weights."""
    fmT = np.ascontiguousarray(featuremap.transpose(1, 2, 0)).astype(np.float32)
    ys = np.arange(H)
    y2 = np.stack([ys, np.minimum(ys + 1, H - 1)], 1)            # [512, 2]
    xs = (np.arange(64)[:, None] * 8 + np.arange(16)[None, :])   # [64, 16]
    xs = np.minimum(xs, W - 1)
    # qtab[y, xb, s, r, c]: s-major so x-window selects are contiguous
    qtab = fmT[y2[:, None, None, :], xs[None, :, :, None], :]    # [512,64,16,2,4]
    qtab = qtab.reshape(NROWS, 128).astype(BF16)

    W1, W2, W3, W4, W5 = Ws
    b1, b2, b3, b4, b5 = bs

    # stg feature order: (x, y, z, pad, f0..f3)
    W1a = np.zeros((FPAD, HID), np.float32)
    W1a[0:3] = W1[0:3]
    W1a[4:8] = W1[3:7]

    def blockdiag(Wm, nb):
        fi, fo = Wm.shape
        out = np.zeros((fi * nb, fo * nb), np.float32)
        for b in range(nb):
            out[b * fi:(b + 1) * fi, b * fo:(b + 1) * fo] = Wm
        return out

    w1blk = blockdiag(W1a, 8)                      # [64, 128]
    w1stack = np.concatenate([w1blk, w1blk], 0)    # [128, 128]

    return {
        "qtab": qtab,
        "w1stack": w1stack.astype(BF16),
        "w2blk": blockdiag(W2, 8).astype(BF16),
        "w3blk": blockdiag(W3, 8).astype(BF16),
        "w4blk": blockdiag(W4, 8).astype(BF16),
        "w5blk": blockdiag(W5, 8).astype(BF16),
        "b1blk": np.tile(b1, 8).reshape(P, 1).astype(np.float32),
        "b2blk": np.tile(b2, 8).reshape(P, 1).astype(np.float32),
        "b3blk": np.tile(b3, 8).reshape(P, 1).astype(np.float32),
        "b4blk": np.tile(b4, 8).reshape(P, 1).astype(np.float32),
        "b5blk": np.tile(b5, 8).reshape(24, 1).astype(np.float32),
        "id128": np.eye(P, dtype=np.float32).astype(BF16),
        "id24": np.eye(24, dtype=np.float32),
    }


def build_program(n_slots=M_SLOTS, s_tile=S, mlp=True, gather=True):
    """Build the per-core Bass program (same program for all 8 cores)."""
    import concourse.bass as bass
    import concourse.tile as tile
    from concourse import bacc, mybir

    f32 = mybir.dt.float32
    bf16 = mybir.dt.bfloat16
    i16 = mybir.dt.int16
    i32 = mybir.dt.int32
    u8d = mybir.dt.uint8
    AF = mybir.ActivationFunctionType
    OP = mybir.AluOpType

    assert n_slots % TCH == 0 and s_tile % GS == 0 and s_tile % TCH == 0
    n_pad = P * n_slots
    GNI = 1024                       # idxs per dma_gather call (64 descs per
                                     # engine-lane = the single-packet limit)
    gcalls = P * TCH // GNI          # gather calls per select chunk (8)
    TW = P * TCH // 16               # wrap idx columns per chunk (512)

    # iteration schedule: small head tiles (short ramp to first gather),
    # full s_tile iters, then a progressively-smaller tail (short drain
    # after the final gather)
    tiles = []
    off = 0
    if n_slots > 3 * s_tile and s_tile >= 4 * TCH:
        for st in (TCH, TCH, s_tile - 2 * TCH):
            tiles.append((off, st))
            off += st
    while off < n_slots:
        st = min(s_tile, n_slots - off)
        if n_slots - off == st and st < s_tile and st > 2 * GS:
            # tail: progressively smaller iterations so the pipeline
            # drain after the final gather is minimal
            while st > 2 * GS:
                big = st - 2 * GS
                big -= big % GS
                if big < GS:
                    break
                tiles.append((off, big))
                off += big
                st -= big
            tiles.append((off, GS))
            tiles.append((off + GS, GS))
            off += 2 * GS
            continue
        assert st % TCH == 0 and st % GS == 0
        tiles.append((off, st))
        off += st
    assert off == n_slots, (off, n_slots)

    nc = bacc.Bacc("TRN2", target_bir_lowering=False, debug=False,
                   enable_asserts=False, num_devices=N_CORES,
                   num_swdge_queues=NQ, dynamic_dma_scratch_size=24576)

    xin = nc.dram_tensor("x", [n_pad, 3], f32, kind="ExternalInput").ap()
    qtab = nc.dram_tensor("qtab", [NROWS, 128], bf16, kind="ExternalInput").ap()
    w1stack = nc.dram_tensor("w1stack", [P, P], bf16, kind="ExternalInput").ap()
    w2 = nc.dram_tensor("w2blk", [P, P], bf16, kind="ExternalInput").ap()
    w3 = nc.dram_tensor("w3blk", [P, P], bf16, kind="ExternalInput").ap()
    w4 = nc.dram_tensor("w4blk", [P, P], bf16, kind="ExternalInput").ap()
    w5 = nc.dram_tensor("w5blk", [P, 24], bf16, kind="ExternalInput").ap()
    b1 = nc.dram_tensor("b1blk", [P, 1], f32, kind="ExternalInput").ap()
    b2i = nc.dram_tensor("b2blk", [P, 1], f32, kind="ExternalInput").ap()
    b3i = nc.dram_tensor("b3blk", [P, 1], f32, kind="ExternalInput").ap()
    b4i = nc.dram_tensor("b4blk", [P, 1], f32, kind="ExternalInput").ap()
    b5i = nc.dram_tensor("b5blk", [24, 1], f32, kind="ExternalInput").ap()
    id128 = nc.dram_tensor("id128", [P, P], bf16, kind="ExternalInput").ap()
    id24 = nc.dram_tensor("id24", [24, 24], f32, kind="ExternalInput").ap()
    yout = nc.dram_tensor("y", [n_pad, 3], f32, kind="ExternalOutput").ap()

    # lane p owns rows [p*n_slots, (p+1)*n_slots)  (contiguous HBM runs)
    xv = xin.rearrange("(p s) c -> p s c", p=P)
    yv = yout.rearrange("(p s) c -> p s c", p=P)

    BIGF = float(2 ** 23)

    from contextlib import ExitStack

    with tile.TileContext(nc) as tc, ExitStack() as ctx:
            ep = ctx.enter_context
            consts = ep(tc.tile_pool(name="consts", bufs=1))
            xio = ep(tc.tile_pool(name="xio", bufs=2))
            xwp = ep(tc.tile_pool(name="xw", bufs=1))
            coord2 = ep(tc.tile_pool(name="coord2", bufs=2))
            coord1 = ep(tc.tile_pool(name="coord1", bufs=1))
            wcoord = ep(tc.tile_pool(name="wcoord", bufs=1))
            jidxp = ep(tc.tile_pool(name="jidx", bufs=8))
            gatp = ep(tc.tile_pool(name="gat", bufs=4))
            shiftp = ep(tc.tile_pool(name="shift", bufs=2))
            stagep = ep(tc.tile_pool(name="stage", bufs=2))
            tsbp = ep(tc.tile_pool(name="tsb", bufs=2))
            actsp = ep(tc.tile_pool(name="acts", bufs=3))
            s5p = ep(tc.tile_pool(name="s5", bufs=2))
            ostagep = ep(tc.tile_pool(name="ostage", bufs=4))
            ptr = ep(tc.tile_pool(name="ptr", bufs=2, space="PSUM"))
            pmm = ep(tc.tile_pool(name="pmm", bufs=2, space="PSUM"))
            p5 = ep(tc.tile_pool(name="p5", bufs=1, space="PSUM"))
            dramp = ep(tc.tile_pool(name="dram", bufs=2, space="DRAM"))

            # ---- constants into SBUF
            w1_sb = consts.tile([P, P], bf16, tag="w1")
            w2_sb = consts.tile([P, P], bf16, tag="w2")
            w3_sb = consts.tile([P, P], bf16, tag="w3")
            w4_sb = consts.tile([P, P], bf16, tag="w4")
            w5_sb = consts.tile([P, 24], bf16, tag="w5")
            b1_sb = consts.tile([P, 1], f32, tag="b1")
            b2_sb = consts.tile([P, 1], f32, tag="b2")
            b3_sb = consts.tile([P, 1], f32, tag="b3")
            b4_sb = consts.tile([P, 1], f32, tag="b4")
            b5_sb = consts.tile([24, 1], f32, tag="b5")
            id128_sb = consts.tile([P, P], bf16, tag="id128")
            id24_sb = consts.tile([24, 24], f32, tag="id24")
            cm05 = consts.tile([P, 1], f32, tag="cm05")
            nc.vector.memset(cm05[:], -0.5)
            ones_sb = consts.tile([P, s_tile], u8d, tag="ones")
            nc.vector.memset(ones_sb[:], 1)
            _const_dmas = (
                (w1_sb, w1stack), (w2_sb, w2), (w3_sb, w3), (w4_sb, w4),
                (w5_sb, w5), (b1_sb, b1), (b2_sb, b2i), (b3_sb, b3i),
                (b4_sb, b4i), (b5_sb, b5i), (id128_sb, id128), (id24_sb, id24),
            )

            def floor_exact(pool, fsrc, tagp, full_shape=None):
                """returns AP with floor(fsrc); exact for f in [0, 2^22)."""
                pk, fw = fsrc.shape[0], fsrc.shape[1]
                fs = full_shape or [P, s_tile]
                b_ = pool.tile(fs, f32, tag=f"fb{tagp}", name=f"fb{tagp}")[:pk, :fw]
                nc.vector.tensor_scalar(out=b_, in0=fsrc, scalar1=BIGF,
                                        scalar2=BIGF, op0=OP.add, op1=OP.subtract)
                # comparison temp shared across call sites (uses are disjoint)
                cgt = pool.tile(fs, f32, tag="fcS", name="fcS")[:pk, :fw]
                nc.vector.tensor_tensor(out=cgt, in0=b_, in1=fsrc, op=OP.is_gt)
                nc.vector.tensor_tensor(out=b_, in0=b_, in1=cgt, op=OP.subtract)
                return b_

            qctr = 0  # global gather queue round-robin
            gni_reg = nc.gpsimd.snap(GNI)

            # ======== PROLOGUE: all wrap-layout gather idxs -> DRAM =====
            # partition (ch, q) of the stacked tile holds chunk ch's
            # 16-partition wrap rows; one 128-wide compute pass per s_tile.
            # Precomputing all iterations up front keeps the Pool engine's
            # gather stream free of mid-loop jidx stalls.
            jd_list = [dramp.tile([P, TW], i16, tag=f"jd{i}",
                                       name=f"jd{i}", bufs=1)
                       for i in range(len(tiles))]
            for it, (sl0, st) in enumerate(tiles):
                chunks = st // TCH
                pk = 16 * chunks
                xws = xwp.tile([P, TW, 3], f32, tag="xws", name="xws")
                for ch in range(chunks):
                    xw_src = bass.AP(
                        tensor=xin.tensor,
                        offset=(sl0 + ch * TCH) * 3,
                        ap=[[n_slots * 3, 16], [16 * n_slots * 3, 8],
                            [3, TCH], [1, 3]],
                    )
                    nc.sync.dma_start(
                        out=xws[16 * ch:16 * (ch + 1)].rearrange(
                            "p (tl s) c -> p tl s c", tl=8),
                        in_=xw_src)
                fxw = wcoord.tile([P, TW], f32, tag="fxw", name="fxw")[:pk]
                nc.scalar.activation(out=fxw, in_=xws[:pk, :, 0], func=AF.Relu,
                                     bias=cm05[:pk], scale=float(W))
                fyw = wcoord.tile([P, TW], f32, tag="fyw", name="fyw")[:pk]
                nc.scalar.activation(out=fyw, in_=xws[:pk, :, 1], func=AF.Relu,
                                     bias=cm05[:pk], scale=float(H))
                u8w = wcoord.tile([P, TW], f32, tag="u8w", name="u8w")[:pk]
                nc.vector.tensor_scalar(out=u8w, in0=fxw, scalar1=0.125,
                                        scalar2=None, op0=OP.mult)
                xbw = floor_exact(wcoord, u8w, "xw", full_shape=[P, TW])
                iyw = floor_exact(wcoord, fyw, "yw", full_shape=[P, TW])
                idxf = wcoord.tile([P, TW], f32, tag="idxf", name="idxf")[:pk]
                nc.vector.scalar_tensor_tensor(out=idxf, in0=iyw, scalar=64.0,
                                               in1=xbw, op0=OP.mult, op1=OP.add)
                # cast to int16, permuting t'' = (tl, s) -> t = 8*s + tl
                jidx16 = wcoord.tile([P, TW], i16, tag="jidx16",
                                     name="jidx16")[:pk]
                nc.vector.tensor_copy(
                    out=jidx16.rearrange("p (s tl) -> p tl s", tl=8),
                    in_=idxf.rearrange("p (tl s) -> p tl s", s=TCH))
                nc.sync.dma_start(out=jd_list[it][:pk], in_=jidx16)

            for sb, csrc in _const_dmas:
                nc.sync.dma_start(out=sb[:], in_=csrc)

            for it, (sl0, st) in enumerate(tiles):
                chunks = st // TCH
                groups = st // GS

                # ======== main (point-layout) coordinate pipeline ========
                xt = xio.tile([P, s_tile, 3], f32, tag="xt", name="xt")[:, :st]
                nc.sync.dma_start(out=xt, in_=xv[:, sl0:sl0 + st, :])

                fx = coord1.tile([P, s_tile], f32, tag="fx", name="fx")[:, :st]
                nc.scalar.activation(out=fx, in_=xt[:, :, 0], func=AF.Relu,
                                     bias=cm05[:], scale=float(W))
                fy = coord1.tile([P, s_tile], f32, tag="fy", name="fy")[:, :st]
                nc.scalar.activation(out=fy, in_=xt[:, :, 1], func=AF.Relu,
                                     bias=cm05[:], scale=float(H))

                u8 = coord1.tile([P, s_tile], f32, tag="u8", name="u8")[:, :st]
                nc.vector.tensor_scalar(out=u8, in0=fx, scalar1=0.125,
                                        scalar2=None, op0=OP.mult)
                xbf = floor_exact(coord1, u8, "x")
                u = coord1.tile([P, s_tile], f32, tag="u", name="u")[:, :st]
                nc.vector.scalar_tensor_tensor(out=u, in0=xbf, scalar=-8.0,
                                               in1=fx, op0=OP.mult, op1=OP.add)
                # q bits (f32 masks for predicated selects) + wx
                b2f = coord2.tile([P, s_tile], f32, tag="b2f", name="b2f")[:, :st]
                nc.vector.tensor_scalar(out=b2f, in0=u, scalar1=4.0,
                                        scalar2=None, op0=OP.is_ge)
                u2 = coord1.tile([P, s_tile], f32, tag="u2", name="u2")[:, :st]
                nc.vector.scalar_tensor_tensor(out=u2, in0=b2f, scalar=-4.0,
                                               in1=u, op0=OP.mult, op1=OP.add)
                b1f = coord1.tile([P, s_tile], f32, tag="b1f", name="b1f")[:, :st]
                nc.vector.tensor_scalar(out=b1f, in0=u2, scalar1=2.0,
                                        scalar2=None, op0=OP.is_ge)
                u3 = coord1.tile([P, s_tile], f32, tag="u3", name="u3")[:, :st]
                nc.vector.scalar_tensor_tensor(out=u3, in0=b1f, scalar=-2.0,
                                               in1=u2, op0=OP.mult, op1=OP.add)
                b0f = coord2.tile([P, s_tile], f32, tag="b0f", name="b0f")[:, :st]
                nc.vector.tensor_scalar(out=b0f, in0=u3, scalar1=1.0,
                                        scalar2=None, op0=OP.is_ge)
                wx = coord2.tile([P, s_tile], bf16, tag="wx", name="wx")[:, :st]
                nc.vector.tensor_tensor(out=wx, in0=u3, in1=b0f, op=OP.subtract)
                # monotone ge-masks for the 4-way first select stage (f32
                # 0.0/1.0, used bitcast-as-i32: 1.0f is a nonzero pattern)
                ge2 = coord2.tile([P, s_tile], f32, tag="ge2", name="ge2")[:, :st]
                nc.vector.tensor_scalar(out=ge2, in0=u, scalar1=2.0,
                                        scalar2=None, op0=OP.is_ge)
                ge6 = coord2.tile([P, s_tile], f32, tag="ge6", name="ge6")[:, :st]
                nc.vector.tensor_scalar(out=ge6, in0=u, scalar1=6.0,
                                        scalar2=None, op0=OP.is_ge)

                iyf = floor_exact(coord1, fy, "y")
                wy = coord2.tile([P, s_tile], bf16, tag="wy", name="wy")[:, :st]
                nc.vector.tensor_tensor(out=wy, in0=fy, in1=iyf, op=OP.subtract)

                # jidx: read back replicated to all 8 Q7 core groups (DRAM
                # source APs may have 0-step partition dims); one tile per
                # chunk so each gather only waits on its own slice
                jidx_ch = []
                for ch in range(chunks):
                    jslc = jd_list[it][16 * ch:16 * (ch + 1), :]
                    rep_src = bass.AP(tensor=jslc.tensor, offset=jslc.offset,
                                      ap=[[0, 8]] + list(jslc.ap))
                    jt = jidxp.tile([P, TW], i16, tag="jidx")
                    nc.sync.dma_start(out=jt[:], in_=rep_src)
                    jidx_ch.append(jt)

                # ======== gather + select + lerp per chunk ========
                stg = stagep.tile([P, s_tile, FPAD], bf16, tag="stg", name="stg")[:, :st]
                nc.vector.memset(stg[:, :, 3], 0.0)
                nc.scalar.activation(out=stg[:, :, 0:3], in_=xt,
                                     func=AF.Copy, bias=0.0, scale=1.0)

                # chunks are processed in pairs: both chunks' gathers are
                # issued back-to-back, then both select ladders run -- this
                # halves the Pool<->Vector handshake frequency.
                chpairs = [[c for c in (cb, cb + 1) if c < chunks]
                           for cb in range(0, chunks, 2)]
                for chp in chpairs:
                  G_of = {}
                  for ch in chp:
                    G_of[ch] = gatp.tile([P, TCH, 128], bf16, tag="G", name="G")
                    if gather:
                        gsl = GNI // P       # slots per gather call (8)
                        gw = GNI // 16       # idx cols per gather call (64)
                        for k in range(gcalls):
                            nc.gpsimd.dma_gather(
                                out_ap=G_of[ch][:, k * gsl:(k + 1) * gsl, :],
                                in_ap=qtab,
                                idxs_ap=jidx_ch[ch][:, k * gw:(k + 1) * gw],
                                num_idxs=GNI, num_idxs_reg=gni_reg, elem_size=128,
                                single_packet=(GNI <= 1024),
                                queue_num=qctr % NQ)
                            qctr += 1
                    else:
                        nc.vector.memset(G_of[ch][:], 0.25)

                  for ch in chp:
                    cs = ch * TCH
                    G = G_of[ch]
                    # payload layout (s, r, c): x-window selects are
                    # contiguous slices. 4-way first stage via a monotone
                    # is_ge mask cascade (last true predicate wins), then a
                    # 2-way second stage; all copies on int32 pairs.
                    onesv = ones_sb[:, 0:TCH, None]
                    g2v = ge2.bitcast(i32)[:, cs:cs + TCH, None]
                    g4v = b2f.bitcast(i32)[:, cs:cs + TCH, None]
                    g6v = ge6.bitcast(i32)[:, cs:cs + TCH, None]
                    m0v = b0f.bitcast(i32)[:, cs:cs + TCH, None]

                    W2t = shiftp.tile([P, TCH, 24], bf16, tag="W2")
                    nc.vector.copy_predicated(
                        out=W2t[:].bitcast(i32),
                        mask=onesv.to_broadcast([P, TCH, 12]),
                        data=G[:].bitcast(i32)[:, :, 0:12])
                    nc.vector.copy_predicated(
                        out=W2t[:].bitcast(i32),
                        mask=g2v.to_broadcast([P, TCH, 12]),
                        data=G[:].bitcast(i32)[:, :, 8:20])
                    nc.vector.copy_predicated(
                        out=W2t[:].bitcast(i32),
                        mask=g4v.to_broadcast([P, TCH, 12]),
                        data=G[:].bitcast(i32)[:, :, 16:28])
                    nc.vector.copy_predicated(
                        out=W2t[:].bitcast(i32),
                        mask=g6v.to_broadcast([P, TCH, 12]),
                        data=G[:].bitcast(i32)[:, :, 24:36])
                    W3t = shiftp.tile([P, TCH, 16], bf16, tag="W3")
                    nc.vector.copy_predicated(
                        out=W3t[:].bitcast(i32),
                        mask=onesv.to_broadcast([P, TCH, 8]),
                        data=W2t[:].bitcast(i32)[:, :, 0:8])
                    nc.vector.copy_predicated(
                        out=W3t[:].bitcast(i32),
                        mask=m0v.to_broadcast([P, TCH, 8]),
                        data=W2t[:].bitcast(i32)[:, :, 4:12])

                    # lerp x then y -> staging features (all contiguous)
                    wxv = wx[:, cs:cs + TCH, None].to_broadcast([P, TCH, 8])
                    wyv = wy[:, cs:cs + TCH, None].to_broadcast([P, TCH, 4])
                    d = shiftp.tile([P, TCH, 8], bf16, tag="d")
                    nc.vector.tensor_tensor(out=d[:], in0=W3t[:, :, 8:16],
                                            in1=W3t[:, :, 0:8], op=OP.subtract)
                    nc.vector.tensor_tensor(out=d[:], in0=d[:], in1=wxv, op=OP.mult)
                    nc.vector.tensor_tensor(out=d[:], in0=W3t[:, :, 0:8],
                                            in1=d[:], op=OP.add)
                    e = shiftp.tile([P, TCH, 4], bf16, tag="e")
                    nc.vector.tensor_tensor(out=e[:], in0=d[:, :, 4:8],
                                            in1=d[:, :, 0:4], op=OP.subtract)
                    nc.vector.tensor_tensor(out=e[:], in0=e[:], in1=wyv, op=OP.mult)
                    nc.vector.tensor_tensor(out=stg[:, cs:cs + TCH, 4:8],
                                            in0=d[:, :, 0:4], in1=e[:], op=OP.add)

                stg_flat = stg.rearrange("p s f -> p (s f)")

                if not mlp:
                    ost = ostagep.tile([P, s_tile, 3], f32, tag="ostd",
                                       name="ostd")[:, :st]
                    nc.scalar.activation(out=ost, in_=stg[:, :, 4:7],
                                         func=AF.Copy, bias=0.0, scale=1.0)
                    nc.sync.dma_start(out=yv[:, sl0:sl0 + st, :], in_=ost)
                    continue

                # ======== MLP groups (GS slots = 8192 points each) ========
                # Groups are processed in pairs with layers interleaved so
                # each activation issues right after the OTHER group's
                # matmul -- the ~2us PE->ACT semaphore latency hides behind
                # the sibling's work instead of stalling the Scalar stream.
                for gp in range(0, groups, 2):
                    gpair = [g for g in (gp, gp + 1) if g < groups]
                    tsb_of = {}
                    for g in gpair:
                        t_ps = ptr.tile([P, 4, P], bf16, tag="tp")
                        for c4 in range(4):
                            base = (g * GS + c4 * 16) * FPAD
                            nc.tensor.transpose(out=t_ps[:, c4, :],
                                                in_=stg_flat[:, base:base + P],
                                                identity=id128_sb[:])
                        tsb_of[g] = tsbp.tile([P, 4, P], bf16, tag="tsb", name="tsb")
                        nc.scalar.activation(out=tsb_of[g][:], in_=t_ps[:],
                                             func=AF.Copy, bias=0.0, scale=1.0)

                    # L1: the 4 c4-blocks are contiguous in both rhs and
                    # psum -> one wide matmul per 64-row half.
                    ps_of = {}
                    for g in gpair:
                        ps = pmm.tile([P, 1024], f32, tag="ps", name="ps")
                        for half in range(2):
                            nc.tensor.matmul(
                                out=ps[:, half * 512:(half + 1) * 512],
                                lhsT=w1_sb[half * 64:(half + 1) * 64, :],
                                rhs=tsb_of[g][half * 64:(half + 1) * 64].rearrange(
                                    "p c4 l -> p (c4 l)"),
                                start=True, stop=True)
                        ps_of[g] = ps
                    h_of = {}
                    for g in gpair:
                        h_of[g] = actsp.tile([P, 1024], bf16, tag="h", name="h")
                        nc.scalar.activation(out=h_of[g][:], in_=ps_of[g][:],
                                             func=AF.Relu, bias=b1_sb[:], scale=1.0)

                    for w_sb, bias_sb in ((w2_sb, b2_sb), (w3_sb, b3_sb), (w4_sb, b4_sb)):
                        for g in gpair:
                            ps = pmm.tile([P, 1024], f32, tag="ps", name="ps")
                            nc.tensor.matmul(out=ps[:, 0:512], lhsT=w_sb[:],
                                             rhs=h_of[g][:, 0:512],
                                             start=True, stop=True)
                            nc.tensor.matmul(out=ps[:, 512:1024], lhsT=w_sb[:],
                                             rhs=h_of[g][:, 512:1024],
                                             start=True, stop=True)
                            ps_of[g] = ps
                        for g in gpair:
                            h_of[g] = actsp.tile([P, 1024], bf16, tag="h", name="h")
                            nc.scalar.activation(out=h_of[g][:], in_=ps_of[g][:],
                                                 func=AF.Relu, bias=bias_sb[:],
                                                 scale=1.0)

                    s5_of = {}
                    for g in gpair:
                        ps5 = p5.tile([24, 1024], f32, tag="ps5", name="ps5")
                        nc.tensor.matmul(out=ps5[:, 0:512], lhsT=w5_sb[:],
                                         rhs=h_of[g][:, 0:512],
                                         start=True, stop=True)
                        nc.tensor.matmul(out=ps5[:, 512:1024], lhsT=w5_sb[:],
                                         rhs=h_of[g][:, 512:1024],
                                         start=True, stop=True)
                        s5_of[g] = s5p.tile([24, 1024], f32, tag="s5", name="s5")
                        nc.scalar.activation(out=s5_of[g][:], in_=ps5[:],
                                             func=AF.Identity, bias=b5_sb[:],
                                             scale=1.0)

                    for g in gpair:
                        s5 = s5_of[g]
                        u_ps = ptr.tile([P, 8, 24], f32, tag="tp")
                        for ui in range(2):
                            for c4 in range(4):
                                nc.tensor.transpose(
                                    out=u_ps[:, c4 * 2 + ui, :],
                                    in_=s5[:, ui * 512 + c4 * P: ui * 512 + (c4 + 1) * P],
                                    identity=id24_sb[:])
                        uv = u_ps.rearrange("p k (b c) -> p k b c", c=3)
                        ost = ostagep.tile([P, GS, 3], f32, tag="ost", name="ost")
                        ostg = ost.rearrange("p (k b) d -> p k b d", k=8)
                        nc.scalar.activation(out=ostg, in_=uv,
                                             func=AF.Copy, bias=0.0, scale=1.0)
                        # issue the store from the Scalar sequencer (HWDGE on
                        # TRN2) so it never head-of-line blocks the Sync
                        # queue's jidx prefetches behind the MLP
                        nc.scalar.dma_start(
                            out=yv[:, sl0 + g * GS:sl0 + (g + 1) * GS, :], in_=ost)

    nc.compile()
    return nc


_PROGRAM_CACHE = {}


def _get_program(n_slots, s_tile):
    key = (n_slots, s_tile)
    if key not in _PROGRAM_CACHE:
        _PROGRAM_CACHE[key] = build_program(n_slots, s_tile)
    return _PROGRAM_CACHE[key]


def make_in_maps(x_full, consts, n_slots=M_SLOTS, n_cores=N_CORES):
    n_pad = P * n_slots
    per = x_full.shape[0] // n_cores
    in_maps = []
    for c in range(n_cores):
        xpad = np.zeros((n_pad, 3), np.float32)
        xpad[:per] = x_full[c * per:(c + 1) * per]
        in_maps.append({"x": xpad, **{k: np.ascontiguousarray(v)
                                      for k, v in consts.items()}})
    return in_maps


def kernel(**inputs):
    from concourse import bass_utils
    from concourse.bass_interp import get_hw_module

    x = np.asarray(inputs["x"], dtype=np.float32)
    fm = np.asarray(inputs["featuremap"], dtype=np.float32)
    Ws = [np.asarray(inputs[f"W{i}"], dtype=np.float32) for i in range(1, 6)]
    bs = [np.asarray(inputs[f"b{i}"], dtype=np.float32) for i in range(1, 6)]

    consts = _build_host_constants(fm, Ws, bs)
    n = x.shape[0]
    assert n == N_TOTAL, n
    per = n // N_CORES

    nc = _get_program(M_SLOTS, S)
    old_m = nc.m
    nc.m = get_hw_module(nc.m)
    try:
        in_maps = make_in_maps(x, consts)
        res = bass_utils.run_bass_kernel_spmd(nc, in_maps,
                                              core_ids=list(range(N_CORES)))
    finally:
        nc.m = old_m
    outs = [r["y"][:per] for r in res.results]
    return np.concatenate(outs, axis=0).astype(np.float32)


if __name__ == "__main__":
    build_program(256, 128)
    print("small program built OK")

